# revision 4
# baseline (speedup 1.0000x reference)
"""LoRA-wrapped Linear (per-batch expert routing) on 8 TRN2 NeuronCores.

out[b] = x[b] @ W.T + bias + SCALING * ((x[b] @ la[b].T) @ lb[b].T)
  with la = lora_a[expert_ids[b]], lb = lora_b[expert_ids[b]]

Sharding: data-parallel over batch B=8 -> one batch element per core.
Host pre-work: per-core gather of the expert's LoRA matrices, transposes so
the contraction dim (d_in) lands on SBUF partitions, bf16 cast for 1 cyc/row
PE throughput, fold SCALING into lb and append bias as a 17th LoRA rank
(delta' = [inter, 1] @ [[2*lb.T], [bias]] = 2*delta + bias), so the whole
output tile is accumulated in PSUM by matmuls alone.

Per-core device kernel (S=2048 seq rows, K=4096 contraction, N=4096 out cols):
  - x.T block [4096, 512] resident in SBUF (4 blocks), W.T streamed 512-col
    chunks; 32 k-tile matmuls (N=512 moving) accumulate each [128, 512] PSUM
    tile, then one K=17 matmul adds LoRA-delta + bias into the same bank.
"""

from contextlib import ExitStack

import ml_dtypes
import numpy as np

SCALING = 32.0 / 16.0
B, S, D_IN, D_OUT, R, E = 8, 2048, 4096, 4096, 16, 8

KT = 128  # contraction tile (PE partition dim)
S_SUB = 128  # output-tile partition dim (seq rows)


def build_nc(
    seq=S,
    d_in=D_IN,
    d_out=D_OUT,
    r=R,
    m_blk=512,
    o_chunk=512,
    compute_dt="bfloat16",
    w_bufs=12,
):
    import concourse.mybir as mybir
    import concourse.tile as tile
    from concourse import bacc

    cdt = getattr(mybir.dt, compute_dt)
    f32 = mybir.dt.float32

    nc = bacc.Bacc("TRN2", target_bir_lowering=False, debug=False, enable_asserts=False)
    xT = nc.dram_tensor("xT", [d_in, seq], cdt, kind="ExternalInput").ap()
    wT = nc.dram_tensor("wT", [d_in, d_out], cdt, kind="ExternalInput").ap()
    laT = nc.dram_tensor("laT", [d_in, r], cdt, kind="ExternalInput").ap()
    lbTb = nc.dram_tensor("lbTb", [r + 1, d_out], cdt, kind="ExternalInput").ap()
    out = nc.dram_tensor("out", [seq, d_out], f32, kind="ExternalOutput").ap()

    n_k = d_in // KT
    n_blk = seq // m_blk
    n_s = m_blk // S_SUB
    n_o = d_out // o_chunk
    i_w = min(512, m_blk)  # interT moving width

    with tile.TileContext(nc) as tc, ExitStack() as ctx:
        xpool = ctx.enter_context(tc.tile_pool(name="x", bufs=2 * n_k))
        wpool = ctx.enter_context(tc.tile_pool(name="w", bufs=w_bufs))
        lapool = ctx.enter_context(tc.tile_pool(name="la", bufs=n_k))
        lbpool = ctx.enter_context(tc.tile_pool(name="lb", bufs=1))
        isbpool = ctx.enter_context(tc.tile_pool(name="isb", bufs=2 * (m_blk // i_w)))
        osbpool = ctx.enter_context(tc.tile_pool(name="osb", bufs=4))
        ipsum = ctx.enter_context(tc.tile_pool(name="ipsum", bufs=2, space="PSUM"))
        opsum = ctx.enter_context(tc.tile_pool(name="opsum", bufs=6, space="PSUM"))

        la_t = []
        for k in range(n_k):
            t = lapool.tile([KT, r], cdt, tag="la", name=f"la{k}")
            nc.sync.dma_start(t[:], laT[k * KT : (k + 1) * KT, :])
            la_t.append(t)
        lb_t = lbpool.tile([r + 1, d_out], cdt, tag="lb", name="lbt")
        nc.sync.dma_start(lb_t[:], lbTb[:])

        for blk in range(n_blk):
            s0 = blk * m_blk
            xt = []
            for k in range(n_k):
                t = xpool.tile([KT, m_blk], cdt, tag="x", name=f"x{blk}_{k}")
                nc.sync.dma_start(t[:], xT[k * KT : (k + 1) * KT, s0 : s0 + m_blk])
                xt.append(t)

            # interT[r, m_blk] = la @ x_blk.T, then bf16 + ones row for the
            # K=r+1 delta/bias matmul.
            it_list = []
            for sb in range(m_blk // i_w):
                ipt = ipsum.tile([r, i_w], f32, tag="ipsum", name=f"ip{blk}_{sb}")
                for k in range(n_k):
                    nc.tensor.matmul(
                        ipt[:],
                        la_t[k][:],
                        xt[k][:, sb * i_w : (sb + 1) * i_w],
                        start=(k == 0),
                        stop=(k == n_k - 1),
                    )
                it = isbpool.tile([r + 1, i_w], cdt, tag="isb", name=f"it{blk}_{sb}")
                nc.vector.memset(it[:], 1.0)
                nc.vector.tensor_copy(it[0:r, :], ipt[:])
                it_list.append(it)

            for o in range(n_o):
                o0 = o * o_chunk
                pts = [
                    opsum.tile(
                        [S_SUB, o_chunk], f32, tag="opsum", name=f"op{blk}_{o}_{s}"
                    )
                    for s in range(n_s)
                ]
                for k in range(n_k):
                    w = wpool.tile([KT, o_chunk], cdt, tag="w", name=f"w{blk}_{o}_{k}")
                    nc.sync.dma_start(w[:], wT[k * KT : (k + 1) * KT, o0 : o0 + o_chunk])
                    for s in range(n_s):
                        nc.tensor.matmul(
                            pts[s][:],
                            xt[k][:, s * S_SUB : (s + 1) * S_SUB],
                            w[:],
                            start=(k == 0),
                            stop=False,
                        )
                for s in range(n_s):
                    gs = s * S_SUB
                    it = it_list[gs // i_w]
                    il = gs % i_w
                    nc.tensor.matmul(
                        pts[s][:],
                        it[:, il : il + S_SUB],
                        lb_t[:, o0 : o0 + o_chunk],
                        start=False,
                        stop=True,
                    )
                    ot = osbpool.tile([S_SUB, o_chunk], f32, tag="osb", name=f"ot{blk}_{o}_{s}")
                    nc.vector.tensor_copy(ot[:], pts[s][:])
                    nc.sync.dma_start(
                        out[s0 + gs : s0 + gs + S_SUB, o0 : o0 + o_chunk], ot[:]
                    )

    nc.compile()
    return nc


def make_in_maps(x, expert_ids, W, b, lora_a, lora_b, np_cdt=ml_dtypes.bfloat16):
    """Host-side shard prep: one in_map per core (= per batch element)."""
    wT = np.ascontiguousarray(W.T).astype(np_cdt)
    eids = np.asarray(expert_ids).astype(np.int64)
    in_maps = []
    for c in range(x.shape[0]):
        e = int(eids[c])
        xT = np.ascontiguousarray(x[c].T).astype(np_cdt)
        laT = np.ascontiguousarray(lora_a[e].T).astype(np_cdt)
        lbT = (SCALING * lora_b[e].T).astype(np.float32)
        lbTb = np.concatenate([lbT, b[None, :].astype(np.float32)], axis=0).astype(
            np_cdt
        )
        in_maps.append({"xT": xT, "wT": wT, "laT": laT, "lbTb": lbTb})
    return in_maps


_NC_CACHE = {}


def kernel(x, expert_ids, W, b, lora_a, lora_b):
    from concourse.bass_utils import run_bass_kernel_spmd

    x = np.asarray(x)
    if "nc" not in _NC_CACHE:
        _NC_CACHE["nc"] = build_nc()
    nc = _NC_CACHE["nc"]
    in_maps = make_in_maps(x, expert_ids, W, b, lora_a, lora_b)
    res = run_bass_kernel_spmd(nc, in_maps, core_ids=list(range(B))).results
    return np.stack([res[c]["out"] for c in range(B)], axis=0)


# revision 7
# speedup vs baseline: 10925.2206x; 10925.2206x over previous
"""LoRA-wrapped Linear (per-batch expert routing) on 8 TRN2 NeuronCores.

out[b] = x[b] @ W.T + bias + SCALING * ((x[b] @ la[b].T) @ lb[b].T)
  with la = lora_a[expert_ids[b]], lb = lora_b[expert_ids[b]]

Sharding: data-parallel over batch B=8 -> one batch element per core.
Host pre-work: per-core gather of the expert's LoRA matrices, transposes so
the contraction dim (d_in) lands on SBUF partitions, bf16 cast for 1 cyc/row
PE throughput, fold SCALING into lb and append bias as a 17th LoRA rank
(delta' = [inter, 1] @ [[2*lb.T], [bias]] = 2*delta + bias), so the whole
output tile is accumulated in PSUM by matmuls alone.

Per-core device kernel (S=2048 seq rows, K=4096 contraction, N=4096 out cols):
  - x.T block [4096, 512] resident in SBUF (4 blocks), W.T streamed 512-col
    chunks; 32 k-tile matmuls (N=512 moving) accumulate each [128, 512] PSUM
    tile, then one K=17 matmul adds LoRA-delta + bias into the same bank.
"""

from contextlib import ExitStack

import ml_dtypes
import numpy as np

SCALING = 32.0 / 16.0
B, S, D_IN, D_OUT, R, E = 8, 2048, 4096, 4096, 16, 8

KT = 128  # contraction tile (PE partition dim)
S_SUB = 128  # output-tile partition dim (seq rows)


def build_nc(
    seq=S,
    d_in=D_IN,
    d_out=D_OUT,
    r=R,
    m_blk=512,
    o_chunk=512,
    compute_dt="bfloat16",
    w_bufs=12,
    passes=1,
):
    import concourse.mybir as mybir
    import concourse.tile as tile
    from concourse import bacc

    cdt = getattr(mybir.dt, compute_dt)
    f32 = mybir.dt.float32

    nc = bacc.Bacc("TRN2", target_bir_lowering=False, debug=False, enable_asserts=False)
    xT = nc.dram_tensor("xT", [d_in, seq], cdt, kind="ExternalInput").ap()
    wT = nc.dram_tensor("wT", [d_in, d_out], cdt, kind="ExternalInput").ap()
    laT = nc.dram_tensor("laT", [d_in, r], cdt, kind="ExternalInput").ap()
    lbTb = nc.dram_tensor("lbTb", [r + 1, d_out], cdt, kind="ExternalInput").ap()
    out = nc.dram_tensor("out", [seq, d_out], f32, kind="ExternalOutput").ap()

    n_k = d_in // KT
    n_blk = seq // m_blk
    n_s = m_blk // S_SUB
    n_o = d_out // o_chunk
    i_w = min(512, m_blk)  # interT moving width

    with tile.TileContext(nc) as tc, ExitStack() as ctx:
        xpool = ctx.enter_context(tc.tile_pool(name="x", bufs=2 * n_k))
        wpool = ctx.enter_context(tc.tile_pool(name="w", bufs=w_bufs))
        lapool = ctx.enter_context(tc.tile_pool(name="la", bufs=n_k))
        lbpool = ctx.enter_context(tc.tile_pool(name="lb", bufs=1))
        isbpool = ctx.enter_context(tc.tile_pool(name="isb", bufs=2 * (m_blk // i_w)))
        osbpool = ctx.enter_context(tc.tile_pool(name="osb", bufs=4))
        ipsum = ctx.enter_context(tc.tile_pool(name="ipsum", bufs=2, space="PSUM"))
        opsum = ctx.enter_context(tc.tile_pool(name="opsum", bufs=6, space="PSUM"))

        la_t = []
        for k in range(n_k):
            t = lapool.tile([KT, r], cdt, tag="la", name=f"la{k}")
            nc.sync.dma_start(t[:], laT[k * KT : (k + 1) * KT, :])
            la_t.append(t)
        lb_t = lbpool.tile([r + 1, d_out], cdt, tag="lb", name="lbt")
        nc.sync.dma_start(lb_t[:], lbTb[:])

        for p, blk in ((p, blk) for p in range(passes) for blk in range(n_blk)):
            s0 = blk * m_blk
            xt = []
            for k in range(n_k):
                t = xpool.tile([KT, m_blk], cdt, tag="x", name=f"x{p}_{blk}_{k}")
                nc.sync.dma_start(t[:], xT[k * KT : (k + 1) * KT, s0 : s0 + m_blk])
                xt.append(t)

            # interT[r, m_blk] = la @ x_blk.T, then bf16 + ones row for the
            # K=r+1 delta/bias matmul.
            it_list = []
            for sb in range(m_blk // i_w):
                ipt = ipsum.tile([r, i_w], f32, tag="ipsum", name=f"ip{p}_{blk}_{sb}")
                for k in range(n_k):
                    nc.tensor.matmul(
                        ipt[:],
                        la_t[k][:],
                        xt[k][:, sb * i_w : (sb + 1) * i_w],
                        start=(k == 0),
                        stop=(k == n_k - 1),
                    )
                it = isbpool.tile([r + 1, i_w], cdt, tag="isb", name=f"it{p}_{blk}_{sb}")
                nc.vector.memset(it[:], 1.0)
                nc.vector.tensor_copy(it[0:r, :], ipt[:])
                it_list.append(it)

            for o in range(n_o):
                o0 = o * o_chunk
                pts = [
                    opsum.tile(
                        [S_SUB, o_chunk], f32, tag="opsum", name=f"op{p}_{blk}_{o}_{s}"
                    )
                    for s in range(n_s)
                ]
                for k in range(n_k):
                    w = wpool.tile([KT, o_chunk], cdt, tag="w", name=f"w{p}_{blk}_{o}_{k}")
                    nc.sync.dma_start(w[:], wT[k * KT : (k + 1) * KT, o0 : o0 + o_chunk])
                    for s in range(n_s):
                        nc.tensor.matmul(
                            pts[s][:],
                            xt[k][:, s * S_SUB : (s + 1) * S_SUB],
                            w[:],
                            start=(k == 0),
                            stop=False,
                        )
                for s in range(n_s):
                    gs = s * S_SUB
                    it = it_list[gs // i_w]
                    il = gs % i_w
                    nc.tensor.matmul(
                        pts[s][:],
                        it[:, il : il + S_SUB],
                        lb_t[:, o0 : o0 + o_chunk],
                        start=False,
                        stop=True,
                    )
                    ot = osbpool.tile([S_SUB, o_chunk], f32, tag="osb", name=f"ot{p}_{blk}_{o}_{s}")
                    nc.vector.tensor_copy(ot[:], pts[s][:])
                    nc.sync.dma_start(
                        out[s0 + gs : s0 + gs + S_SUB, o0 : o0 + o_chunk], ot[:]
                    )

    nc.compile()
    return nc


def make_in_maps(x, expert_ids, W, b, lora_a, lora_b, np_cdt=ml_dtypes.bfloat16):
    """Host-side shard prep: one in_map per core (= per batch element)."""
    wT = np.ascontiguousarray(W.T).astype(np_cdt)
    eids = np.asarray(expert_ids).astype(np.int64)
    in_maps = []
    for c in range(x.shape[0]):
        e = int(eids[c])
        xT = np.ascontiguousarray(x[c].T).astype(np_cdt)
        laT = np.ascontiguousarray(lora_a[e].T).astype(np_cdt)
        lbT = (SCALING * lora_b[e].T).astype(np.float32)
        lbTb = np.concatenate([lbT, b[None, :].astype(np.float32)], axis=0).astype(
            np_cdt
        )
        in_maps.append({"xT": xT, "wT": wT, "laT": laT, "lbTb": lbTb})
    return in_maps


_NC_CACHE = {}


def kernel(x, expert_ids, W, b, lora_a, lora_b):
    from concourse.bass_utils import run_bass_kernel_spmd

    x = np.asarray(x)
    if "nc" not in _NC_CACHE:
        _NC_CACHE["nc"] = build_nc()
    nc = _NC_CACHE["nc"]
    in_maps = make_in_maps(x, expert_ids, W, b, lora_a, lora_b)
    res = run_bass_kernel_spmd(nc, in_maps, core_ids=list(range(B))).results
    return np.stack([res[c]["out"] for c in range(B)], axis=0)


# revision 16
# speedup vs baseline: 13761.0854x; 1.2596x over previous
"""LoRA-wrapped Linear (per-batch expert routing) on 8 TRN2 NeuronCores.

out[b] = x[b] @ W.T + bias + SCALING * ((x[b] @ la[b].T) @ lb[b].T)
  with la = lora_a[expert_ids[b]], lb = lora_b[expert_ids[b]]

Sharding: data-parallel over batch B=8 -> one batch element per core.
Host pre-work: per-core gather of the expert's LoRA matrices, transposes so
the contraction dim (d_in) lands on SBUF partitions, bf16 cast for 1 cyc/row
PE throughput, fold SCALING into lb and append bias as a 17th LoRA rank
(delta' = [inter, 1] @ [[2*lb.T], [bias]] = 2*delta + bias), so the whole
output tile is accumulated in PSUM by matmuls alone.

Per-core device kernel (S=2048 seq rows, K=4096 contraction, N=4096 out cols):
  - x.T block [4096, 512] resident in SBUF (4 blocks), W.T streamed 512-col
    chunks; 32 k-tile matmuls (N=512 moving) accumulate each [128, 512] PSUM
    tile, then one K=17 matmul adds LoRA-delta + bias into the same bank.
"""

from contextlib import ExitStack

import ml_dtypes
import numpy as np

SCALING = 32.0 / 16.0
B, S, D_IN, D_OUT, R, E = 8, 2048, 4096, 4096, 16, 8

KT = 128  # contraction tile (PE partition dim)
S_SUB = 128  # output-tile partition dim (seq rows)


def build_nc(
    seq=S,
    d_in=D_IN,
    d_out=D_OUT,
    r=R,
    m_blk=512,
    o_chunk=512,
    compute_dt="bfloat16",
    w_bufs=6,
    passes=1,
    opsum_bufs=6,
    ipsum_bufs=2,
):
    import concourse.mybir as mybir
    import concourse.tile as tile
    from concourse import bacc

    cdt = getattr(mybir.dt, compute_dt)
    f32 = mybir.dt.float32

    nc = bacc.Bacc("TRN2", target_bir_lowering=False, debug=False, enable_asserts=False)
    xT = nc.dram_tensor("xT", [d_in, seq], cdt, kind="ExternalInput").ap()
    wT = nc.dram_tensor("wT", [d_in, d_out], cdt, kind="ExternalInput").ap()
    laT = nc.dram_tensor("laT", [d_in, r], cdt, kind="ExternalInput").ap()
    lbTb = nc.dram_tensor("lbTb", [r + 1, d_out], cdt, kind="ExternalInput").ap()
    out = nc.dram_tensor("out", [seq, d_out], f32, kind="ExternalOutput").ap()

    n_k = d_in // KT
    KG = min(8, n_k)  # k-tiles per W-group DMA
    assert n_k % KG == 0
    n_blk = seq // m_blk
    n_s = m_blk // S_SUB
    n_o = d_out // o_chunk
    i_w = min(512, m_blk)  # interT moving width

    with tile.TileContext(nc) as tc, ExitStack() as ctx:
        xpool = ctx.enter_context(tc.tile_pool(name="x", bufs=2 * n_k))
        wpool = ctx.enter_context(tc.tile_pool(name="w", bufs=w_bufs))
        lapool = ctx.enter_context(tc.tile_pool(name="la", bufs=n_k))
        lbpool = ctx.enter_context(tc.tile_pool(name="lb", bufs=1))
        isbpool = ctx.enter_context(tc.tile_pool(name="isb", bufs=2 * (m_blk // i_w)))
        osbpool = ctx.enter_context(tc.tile_pool(name="osb", bufs=3))
        ipsum = ctx.enter_context(tc.tile_pool(name="ipsum", bufs=ipsum_bufs, space="PSUM"))
        opsum = ctx.enter_context(tc.tile_pool(name="opsum", bufs=opsum_bufs, space="PSUM"))

        blk_seq = [(p, blk) for p in range(passes) for blk in range(n_blk)]

        def issue_x(p, blk, k):
            s0 = blk * m_blk
            t = xpool.tile([KT, m_blk], cdt, tag="x", name=f"x{p}_{blk}_{k}")
            nc.sync.dma_start(t[:], xT[k * KT : (k + 1) * KT, s0 : s0 + m_blk])
            return t

        def issue_w_grp(p, blk, o, kg, kg_size=None):
            # one DMA for kg_size k-tiles: [128, kg_size, o_chunk] <- wT rows
            # kg*kg_size*128..(kg+1)*kg_size*128 (row = k*128 + partition)
            g = kg_size or KG
            o0 = o * o_chunk
            w = wpool.tile(
                [KT, g, o_chunk], cdt, tag="w", name=f"w{p}_{blk}_{o}_{kg}"
            )
            srcap = wT[kg * g * KT : (kg + 1) * g * KT, o0 : o0 + o_chunk]
            nc.sync.dma_start(w[:], srcap.rearrange("(k p) o -> p k o", p=KT))
            return w

        # Block 0 startup: interleave x-block DMAs with o=0's W-group DMAs so
        # the first base k-loop is paced by (w_grp, x*KG) bundles instead of
        # the PE idling behind the whole x block in the DMA queue.
        xt_pre = {}
        w0_pre = []
        p0, b0 = blk_seq[0]
        xt_pre[(p0, b0)] = []
        KG0 = KG  # finer first-chunk groups did not help in the cost model
        for kg in range(n_k // KG0):
            w0_pre.append(issue_w_grp(p0, b0, 0, kg, KG0))
            for k in range(kg * KG0, (kg + 1) * KG0):
                xt_pre[(p0, b0)].append(issue_x(p0, b0, k))

        la_t = []
        for k in range(n_k):
            t = lapool.tile([KT, r], cdt, tag="la", name=f"la{k}")
            nc.sync.dma_start(t[:], laT[k * KT : (k + 1) * KT, :])
            la_t.append(t)
        lb_t = lbpool.tile([r + 1, d_out], cdt, tag="lb", name="lbt")
        nc.sync.dma_start(lb_t[:], lbTb[:])

        for bi, (p, blk) in enumerate(blk_seq):
            s0 = blk * m_blk
            first = bi == 0
            xt = xt_pre.pop((p, blk)) if (p, blk) in xt_pre else [
                issue_x(p, blk, k) for k in range(n_k)
            ]

            # interT[r, m_blk] = la @ x_blk.T (bf16 + ones row for the K=r+1
            # delta/bias matmul). For block 0 this is emitted after o=0's base
            # k-loop (x arrives DMA-paced there; inter would stall the PE).
            it_list = [None] * (m_blk // i_w)

            def compute_inter():
                for sb in range(m_blk // i_w):
                    it = isbpool.tile(
                        [r + 1, i_w], cdt, tag="isb", name=f"it{p}_{blk}_{sb}"
                    )
                    nc.vector.memset(it[:], 1.0)
                    ipt = ipsum.tile(
                        [r, i_w], f32, tag="ipsum", name=f"ip{p}_{blk}_{sb}"
                    )
                    for k in range(n_k):
                        nc.tensor.matmul(
                            ipt[:],
                            la_t[k][:],
                            xt[k][:, sb * i_w : (sb + 1) * i_w],
                            start=(k == 0),
                            stop=(k == n_k - 1),
                        )
                    nc.vector.tensor_copy(it[0:r, :], ipt[:])
                    it_list[sb] = it

            if not first:
                compute_inter()

            for o in range(n_o):
                o0 = o * o_chunk
                pts = [
                    opsum.tile(
                        [S_SUB, o_chunk], f32, tag="opsum", name=f"op{p}_{blk}_{o}_{s}"
                    )
                    for s in range(n_s)
                ]
                # prefetch next block's x tiles under this block's second-to-
                # last o-chunk so the W stream doesn't starve them at the
                # block boundary.
                nxt = blk_seq[bi + 1] if bi + 1 < len(blk_seq) else None
                prefetch_x = o == max(0, n_o - 2) and nxt is not None
                if prefetch_x:
                    xt_pre[nxt] = []
                g = KG0 if (first and o == 0) else KG
                for kg in range(n_k // g):
                    wg = (
                        w0_pre[kg]
                        if (first and o == 0)
                        else issue_w_grp(p, blk, o, kg)
                    )
                    for ki in range(g):
                        k = kg * g + ki
                        if prefetch_x:
                            xt_pre[nxt].append(issue_x(nxt[0], nxt[1], k))
                        for s in range(n_s):
                            nc.tensor.matmul(
                                pts[s][:],
                                xt[k][:, s * S_SUB : (s + 1) * S_SUB],
                                wg[:, ki, :],
                                start=(k == 0),
                                stop=False,
                            )
                if first and o == 0:
                    compute_inter()
                ot = osbpool.tile(
                    [S_SUB, n_s, o_chunk], f32, tag="osb", name=f"ot{p}_{blk}_{o}"
                )
                for s in range(n_s):
                    gs = s * S_SUB
                    it = it_list[gs // i_w]
                    il = gs % i_w
                    nc.tensor.matmul(
                        pts[s][:],
                        it[:, il : il + S_SUB],
                        lb_t[:, o0 : o0 + o_chunk],
                        start=False,
                        stop=True,
                    )
                    nc.vector.tensor_copy(ot[:, s, :], pts[s][:])
                dst = out[s0 : s0 + m_blk, o0 : o0 + o_chunk]
                nc.sync.dma_start(
                    dst.rearrange("(g q) o -> q g o", q=S_SUB), ot[:]
                )

    nc.compile()
    return nc


def make_in_maps(x, expert_ids, W, b, lora_a, lora_b, np_cdt=ml_dtypes.bfloat16):
    """Host-side shard prep: one in_map per core (= per batch element)."""
    wT = np.ascontiguousarray(W.T).astype(np_cdt)
    eids = np.asarray(expert_ids).astype(np.int64)
    in_maps = []
    for c in range(x.shape[0]):
        e = int(eids[c])
        xT = np.ascontiguousarray(x[c].T).astype(np_cdt)
        laT = np.ascontiguousarray(lora_a[e].T).astype(np_cdt)
        lbT = (SCALING * lora_b[e].T).astype(np.float32)
        lbTb = np.concatenate([lbT, b[None, :].astype(np.float32)], axis=0).astype(
            np_cdt
        )
        in_maps.append({"xT": xT, "wT": wT, "laT": laT, "lbTb": lbTb})
    return in_maps


_NC_CACHE = {}


def kernel(x, expert_ids, W, b, lora_a, lora_b):
    from concourse.bass_utils import run_bass_kernel_spmd

    x = np.asarray(x)
    if "nc" not in _NC_CACHE:
        _NC_CACHE["nc"] = build_nc()
    nc = _NC_CACHE["nc"]
    in_maps = make_in_maps(x, expert_ids, W, b, lora_a, lora_b)
    res = run_bass_kernel_spmd(nc, in_maps, core_ids=list(range(B))).results
    return np.stack([res[c]["out"] for c in range(B)], axis=0)
